# revision 16
# baseline (speedup 1.0000x reference)
"""Sliding-window MHA Trainium2 kernel, sharded over 8 NeuronCores.

Problem (hardcoded): B=2, L=2048, D=1024, H=16 heads (hd=64), window
|i-j| <= 256, fp32 I/O.

Sharding: core = b*4 + g (b in 0..1 batches, g in 0..3 head-groups of 4
heads). Each core: QKV projection for its 4 heads, banded attention, and
a partial output projection (its 256 columns of the head concat). Host
sums the partials, removes the dither correction, and rescales.

Numerics (all fp8 uses are residual-corrected except the single q
requantization):
  - Weights are scaled x16 before e4m3 quantization (their natural scale
    sits in e4m3's subnormal range); activations/V/outputs carry the x16
    factor through and the host divides by 256 at the end.
  - QKV projections: fp8 DoubleRow (contraction 2x128/matmul, 0.5
    cyc/col), 3 terms: x8@w8 + xe8@w8 + x8@we8 (x and w residuals).
  - Scores: fp8 DoubleRow per head; the two z-slots compute
    (k8 + ke8).q8, i.e. the k requantization is corrected in-slot for
    free. ke8 is produced by one extra DVE op per k chain. The q
    requantization is the one uncorrected noise source.
  - AV in fp16 (V carries 16x), both heads of a pair in one PSUM bank
    with per-region start/stop; one reciprocal + one stride-0 broadcast
    normalize per pair.
  - Out-projection in fp16 (o stays 16x; host divides by 16): fp8
    here is too noisy for the 2e-2 gate even dithered.
"""

import numpy as np
import ml_dtypes

import concourse.bacc as bacc
import concourse.mybir as mybir
import concourse.tile as tile
from concourse.bass_utils import run_bass_kernel_spmd
from concourse.masks import make_identity

F32 = mybir.dt.float32
F16 = mybir.dt.float16
F8 = mybir.dt.float8e4
E4M3 = ml_dtypes.float8_e4m3
DR = mybir.MatmulPerfMode.DoubleRow

P = 128
L = 2048
D = 1024
NH = 4          # heads per core
HD = 64
FV = 256        # v feature rows per core
WIN = 256
KB = L // P     # 16 k/token blocks
N_CORES = 8
WS = 16.0       # weight/activation scale before fp8
EXP_SCALE = 0.125 / (WS * WS)

C1_LAG = 3
D_LAG = 4


def _window(kb):
    k0 = kb * P
    qlo = max(0, k0 - WIN)
    qhi = min(L, k0 + P + WIN)
    return qlo, qhi - qlo


def _build_nc():
    nc = bacc.Bacc(
        "TRN2", target_bir_lowering=False, debug=False, num_devices=N_CORES
    )
    x8_d = nc.dram_tensor("x8", [P, 8, L], F8, kind="ExternalInput").ap()
    xe8_d = nc.dram_tensor("xe8", [P, 8, L], F8, kind="ExternalInput").ap()
    wqk_d = nc.dram_tensor("wqk8", [P, 4, 8, P], F8, kind="ExternalInput").ap()
    wqke_d = nc.dram_tensor("wqke8", [P, 4, 8, P], F8, kind="ExternalInput").ap()
    wv_d = nc.dram_tensor("wv8", [P, 8, FV], F8, kind="ExternalInput").ap()
    wve_d = nc.dram_tensor("wve8", [P, 8, FV], F8, kind="ExternalInput").ap()
    wo_d = nc.dram_tensor("wo16", [P, 2, D], F16, kind="ExternalInput").ap()
    bqk_d = nc.dram_tensor("bqk", [P, 4], F32, kind="ExternalInput").ap()
    bv_d = nc.dram_tensor("bv", [1, FV], F32, kind="ExternalInput").ap()
    y_d = nc.dram_tensor("y", [L, D], F16, kind="ExternalOutput").ap()

    with tile.TileContext(nc) as tc:
        _emit(nc, tc, x8_d, xe8_d, wqk_d, wqke_d, wv_d, wve_d, wo_d,
              bqk_d, bv_d, y_d)
    nc.compile()
    return nc


def _emit(nc, tc, x8_d, xe8_d, wqk_d, wqke_d, wv_d, wve_d, wo_d,
          bqk_d, bv_d, y_d):
    import contextlib

    ctx = contextlib.ExitStack()
    with ctx:
        const = ctx.enter_context(tc.tile_pool(name="const", bufs=1))
        w_pool = ctx.enter_context(tc.tile_pool(name="w", bufs=1))
        qk_pool = ctx.enter_context(tc.tile_pool(name="qk", bufs=1))
        v_pool = ctx.enter_context(tc.tile_pool(name="v", bufs=1))
        x_pool = ctx.enter_context(tc.tile_pool(name="x", bufs=1))
        e_pool = ctx.enter_context(tc.tile_pool(name="e", bufs=7))
        oT_pool = ctx.enter_context(tc.tile_pool(name="oT", bufs=1))
        opr_pool = ctx.enter_context(tc.tile_pool(name="opr", bufs=3))
        rr_pool = ctx.enter_context(tc.tile_pool(name="rr", bufs=4))
        ysb_pool = ctx.enter_context(tc.tile_pool(name="ysb", bufs=3))
        spool = ctx.enter_context(tc.tile_pool(name="spsum", bufs=2, space="PSUM"))
        pqpool = ctx.enter_context(tc.tile_pool(name="pqpsum", bufs=2, space="PSUM"))
        otpool = ctx.enter_context(tc.tile_pool(name="otpsum", bufs=2, space="PSUM"))

        # ---- input DMAs (pipeline-ordered) ------------------------------
        bqk_sb = const.tile([P, 4], F32)
        bv_row = const.tile([1, FV], F32)
        wqk_sb = w_pool.tile([P, 4, 8, P], F8)
        wqke_sb = w_pool.tile([P, 4, 8, P], F8)
        x8_sb = x_pool.tile([P, 8, L], F8)
        xe8_sb = x_pool.tile([P, 8, L], F8)
        # startup order: first x chunk, then weights fc by fc, then the
        # rest of x interleaved with v/o weights
        sl0 = slice(0, 512)
        nc.sync.dma_start(x8_sb[:, :, sl0], x8_d[:, :, sl0])
        nc.sync.dma_start(wqk_sb[:, 0, :, :], wqk_d[:, 0, :, :])
        nc.sync.dma_start(wqke_sb[:, 0, :, :], wqke_d[:, 0, :, :])
        nc.sync.dma_start(xe8_sb[:, :, sl0], xe8_d[:, :, sl0])
        nc.sync.dma_start(bqk_sb[:], bqk_d[:])
        nc.sync.dma_start(bv_row[:], bv_d[:])
        wv_sb = w_pool.tile([P, 8, FV], F8)
        wve_sb = w_pool.tile([P, 8, FV], F8)
        wo_sb = w_pool.tile([P, 2, D], F16)
        for fc in (2, 1, 3):
            nc.sync.dma_start(wqk_sb[:, fc, :, :], wqk_d[:, fc, :, :])
            nc.sync.dma_start(wqke_sb[:, fc, :, :], wqke_d[:, fc, :, :])
            if fc == 1:
                nc.sync.dma_start(wv_sb[:], wv_d[:])
                nc.sync.dma_start(wve_sb[:], wve_d[:])
        for t in range(1, 4):
            sl = slice(t * 512, (t + 1) * 512)
            nc.sync.dma_start(x8_sb[:, :, sl], x8_d[:, :, sl])
            nc.sync.dma_start(xe8_sb[:, :, sl], xe8_d[:, :, sl])
            if t == 2:
                nc.sync.dma_start(wo_sb[:], wo_d[:])

        # ---- constants --------------------------------------------------
        ident_f32 = const.tile([P, P], F32)
        make_identity(nc, ident_f32[:])
        ident = const.tile([P, P], F16)
        nc.vector.tensor_copy(ident[:], ident_f32[:])
        bv_bc = const.tile([P, FV], F32)
        nc.gpsimd.partition_broadcast(bv_bc[:], bv_row[:])

        # Q^T (fp8, 16x): [dims(2 heads x 64), cc, token]
        qT8 = qk_pool.tile([P, 2, L], F8)
        # K^T with z-dim: z0 = k8, z1 = requant residual
        kT8 = qk_pool.tile([P, 2, 2, L], F8)
        v_ext = v_pool.tile([P, KB, NH * (HD + 1)], F16)
        nc.vector.memset(
            v_ext[:].rearrange("p b (h c) -> p b h c", h=NH)[:, :, :, HD:],
            1.0)
        oT = oT_pool.tile([P, 2, L], F16)

        # ---- emission helpers -------------------------------------------
        def qk_chain(fc, t, half=None):
            """12 fp8-DR matmuls (x8@w8, xe8@w8, x8@we8), fp8 bias-add;
            for k chains also the requant residual (z1)."""
            if half is None:
                tsl = slice(t * 512, (t + 1) * 512)
            else:
                tsl = slice(t * 512 + half * 256, t * 512 + half * 256 + 256)
            n = tsl.stop - tsl.start
            pq = pqpool.tile([P, 512], F32, tag="pq", name="pq")
            terms = ((x8_sb, wqk_sb), (x8_sb, wqke_sb), (xe8_sb, wqk_sb))
            for i, (xs, ws) in enumerate(terms):
                for d4 in range(4):
                    nc.tensor.matmul(
                        pq[:, 0:n],
                        lhsT=ws[:, fc, 2 * d4:2 * d4 + 2, :],
                        rhs=xs[:, 2 * d4:2 * d4 + 2, tsl],
                        start=(i == 0 and d4 == 0), stop=(i == 2 and d4 == 3),
                        perf_mode=DR,
                    )
            cc = fc % 2
            if fc < 2:
                nc.vector.tensor_scalar_add(
                    qT8[:, cc, tsl], pq[:, 0:n], bqk_sb[:, fc:fc + 1])
            else:
                nc.vector.tensor_scalar_add(
                    kT8[:, cc, 0, tsl], pq[:, 0:n], bqk_sb[:, fc:fc + 1])
                # z1 = (pq + bias) - k8   (requant residual)
                nc.vector.scalar_tensor_tensor(
                    kT8[:, cc, 1, tsl], pq[:, 0:n], bqk_sb[:, fc:fc + 1],
                    kT8[:, cc, 0, tsl],
                    mybir.AluOpType.add, mybir.AluOpType.subtract)

        def v_chain(tb):
            tsl = slice(tb * P, (tb + 1) * P)
            pv = pqpool.tile([P, 512], F32, tag="pq", name="pv")
            terms = ((x8_sb, wv_sb), (x8_sb, wve_sb), (xe8_sb, wv_sb))
            for i, (xs, ws) in enumerate(terms):
                for d4 in range(4):
                    nc.tensor.matmul(
                        pv[:, 0:FV],
                        lhsT=xs[:, 2 * d4:2 * d4 + 2, tsl],
                        rhs=ws[:, 2 * d4:2 * d4 + 2, :],
                        start=(i == 0 and d4 == 0), stop=(i == 2 and d4 == 3),
                        perf_mode=DR,
                    )
            nc.vector.tensor_add(
                v_ext[:, tb, :].rearrange("p (h c) -> p h c", h=NH)[:, :, 0:HD],
                pv[:, 0:FV].rearrange("p (h c) -> p h c", h=NH),
                bv_bc[:].rearrange("p (h c) -> p h c", h=NH),
            )

        def phase_b(h, kb, e_tiles):
            """Scores for one head: fp8-DR, z = (k8, k-residual)."""
            qlo, w = _window(kb)
            if h == 0:
                e4 = e_pool.tile([P, NH, 640], F16, tag="e", name="e4")
                e_tiles[kb] = e4
            else:
                e4 = e_tiles[kb]
            s_ps = spool.tile([P, 1024], F32, tag="s", name="s_ps")
            cc, hh = h // 2, h % 2
            base = 64 * hh
            if w == 640:
                pieces = [(0, 0, 320), (320, 512, 320)]
            else:
                pieces = [(0, 0, w)]
            lhsT = kT8[base:base + 64, cc, :, kb * P:(kb + 1) * P]
            for qoff, poff, pw in pieces:
                rhs = (qT8[base:base + 64, cc, qlo + qoff:qlo + qoff + pw]
                       .unsqueeze(1).broadcast_to([64, 2, pw]))
                nc.tensor.matmul(
                    s_ps[:, poff:poff + pw],
                    lhsT=lhsT, rhs=rhs, start=True, stop=True,
                    perf_mode=DR, skip_group_check=True,
                )
            if w == 640:
                src = s_ps[:].rearrange("p (g c) -> p g c", g=2)[:, :, 0:320]
                dst = e4[:, h, :].rearrange("p (g c) -> p g c", g=2)
            else:
                src = s_ps[:, 0:w]
                dst = e4[:, h, 0:w]
            nc.scalar.activation(
                dst, src, mybir.ActivationFunctionType.Exp, scale=EXP_SCALE)

        def masks(kb, pair, e_tiles):
            e4 = e_tiles[kb]
            qlo, w = _window(kb)
            hs = slice(2 * pair, 2 * pair + 2)
            if kb >= 2:
                nc.gpsimd.affine_select(
                    out=e4[:, hs, 0:P], in_=e4[:, hs, 0:P],
                    compare_op=mybir.AluOpType.is_ge, fill=0.0,
                    base=0, pattern=[[0, 2], [1, P]], channel_multiplier=-1,
                )
            if kb <= KB - 3:
                nc.gpsimd.affine_select(
                    out=e4[:, hs, w - P:w], in_=e4[:, hs, w - P:w],
                    compare_op=mybir.AluOpType.is_ge, fill=0.0,
                    base=0, pattern=[[0, 2], [-1, P]], channel_multiplier=1,
                )

        def phase_c(pair, qt, e_tiles):
            """AV (pair in one bank), recip + stride-0 normalize, fp16
            transpose, dithered fp8 oT write."""
            kbs = range(max(0, qt - 2), min(KB, qt + 3))
            ot = otpool.tile([P, 512], F32, tag="ot", name="ot")
            for hh in range(2):
                h = 2 * pair + hh
                for i, kb in enumerate(kbs):
                    qlo, _ = _window(kb)
                    off = qt * P - qlo
                    nc.tensor.matmul(
                        ot[:, hh * 65:hh * 65 + 65],
                        lhsT=e_tiles[kb][:, h, off:off + P],
                        rhs=v_ext[:, kb, h * 65:h * 65 + 65],
                        start=(i == 0), stop=(i == len(kbs) - 1),
                        skip_group_check=True,
                    )
            o_pair = ot[:, 0:130].rearrange("p (h c) -> p h c", h=2)
            rr = rr_pool.tile([P, 2], F32, tag="rr", name="rr")
            nc.vector.reciprocal(rr[:], o_pair[:, :, HD:HD + 1])
            opr = opr_pool.tile([P, P], F16, tag="opr", name="opr")
            nc.vector.tensor_mul(
                opr[:].rearrange("p (h c) -> p h c", h=2),
                o_pair[:, :, 0:HD],
                rr[:].rearrange("p (h o) -> p h o", h=2).broadcast_to([P, 2, HD]),
            )
            t_ps = ot[:, 256:320].bitcast(F16)
            nc.tensor.transpose(t_ps, opr[:], ident[:])
            nc.vector.tensor_copy(oT[:, pair, qt * P:(qt + 1) * P], t_ps)

        def phase_d(qt, step):
            y_sb = ysb_pool.tile([P, D], F16, tag="ysb", name="y_sb")
            tail = qt >= KB - 2
            late = step >= 10
            for half in range(2):
                y_ps = pqpool.tile([P, 512], F32, tag="pq", name="y_ps")
                hsl = slice(half * 512, (half + 1) * 512)
                for cc in range(2):
                    nc.tensor.matmul(
                        y_ps[:],
                        lhsT=oT[:, cc, qt * P:(qt + 1) * P],
                        rhs=wo_sb[:, cc, hsl],
                        start=(cc == 0), stop=(cc == 1),
                    )
                if tail:
                    # quarter copies on alternating engines + quarter DMAs
                    q0 = slice(half * 512, half * 512 + 256)
                    q1 = slice(half * 512 + 256, half * 512 + 512)
                    nc.vector.tensor_copy(y_sb[:, q0], y_ps[:, 0:256])
                    nc.sync.dma_start(y_d[qt * P:(qt + 1) * P, q0],
                                      y_sb[:, q0])
                    nc.scalar.copy(y_sb[:, q1], y_ps[:, 256:512])
                    nc.sync.dma_start(y_d[qt * P:(qt + 1) * P, q1],
                                      y_sb[:, q1])
                elif late or half == 0:
                    nc.vector.tensor_copy(y_sb[:, hsl], y_ps[:])
                else:
                    nc.scalar.copy(y_sb[:, hsl], y_ps[:])
            if not tail:
                nc.sync.dma_start(y_d[qt * P:(qt + 1) * P, :], y_sb[:])

        # ---- schedule ---------------------------------------------------
        proj_a, proj_b = {}, {}
        proj_a[1] = [lambda: qk_chain(0, 1), lambda: qk_chain(2, 1)]
        proj_b[1] = [lambda: qk_chain(1, 1), lambda: qk_chain(3, 1)]
        proj_a[3] = [lambda: qk_chain(0, 2)]
        proj_a[4] = [lambda: qk_chain(2, 2)]
        proj_a[5] = [lambda: qk_chain(1, 2)]
        proj_a[6] = [lambda: qk_chain(3, 2)]
        proj_a[7] = [lambda: qk_chain(0, 3)]
        proj_b[7] = [lambda: qk_chain(2, 3)]
        proj_a[8] = [lambda: qk_chain(1, 3)]
        proj_a[9] = [lambda: qk_chain(3, 3)]
        # v blocks: emitted in the early slot (before phase_c of the
        # step), respecting wv DMA arrival (~step 2)
        v_sched_a = {3: [2], 4: [3, 4]}
        for s in range(5, KB):
            v_sched_a[s] = [s]
        v_sched_b = {2: [0, 1]}

        # prologue: wave 0 chains
        for fc in (0, 2, 1, 3):
            qk_chain(fc, 0)

        e_tiles = {}
        for step in range(KB + C1_LAG):
            kb = step if step < KB else None
            qt = step - C1_LAG
            qt2 = step - D_LAG
            if kb is not None:
                phase_b(0, kb, e_tiles)
                phase_b(1, kb, e_tiles)
                masks(kb, 0, e_tiles)
            for f in proj_a.get(step, ()):
                f()
            for n in v_sched_a.get(step, ()):
                v_chain(n)
            if qt in range(KB):
                phase_c(0, qt, e_tiles)
            if qt2 in range(KB):
                phase_d(qt2, step)
            if kb is not None:
                phase_b(2, kb, e_tiles)
                phase_b(3, kb, e_tiles)
                masks(kb, 1, e_tiles)
            for f in proj_b.get(step, ()):
                f()
            for n in v_sched_b.get(step, ()):
                v_chain(n)
            if qt in range(KB):
                phase_c(1, qt, e_tiles)
            if qt == KB - 1:
                phase_d(KB - 1, step)


_NC_CACHE = None


def _get_nc():
    global _NC_CACHE
    if _NC_CACHE is None:
        _NC_CACHE = _build_nc()
    return _NC_CACHE


def _fp8_split(a):
    hi = a.astype(E4M3)
    lo = (a - hi.astype(np.float32)).astype(E4M3)
    return hi, lo


def kernel(x, qkv_w, qkv_b, out_w, out_b):
    x = np.asarray(x, dtype=np.float32)
    qkv_w = np.asarray(qkv_w, dtype=np.float32)
    qkv_b = np.asarray(qkv_b, dtype=np.float32)
    out_w = np.asarray(out_w, dtype=np.float32)
    out_b = np.asarray(out_b, dtype=np.float32)
    B = x.shape[0]
    assert x.shape == (B, L, D) and B * 4 == N_CORES

    nc = _get_nc()

    xs = []
    for b in range(B):
        xt = np.ascontiguousarray(x[b].T)            # [D, L]
        x8, xe8 = _fp8_split(xt)
        xs.append((
            np.ascontiguousarray(x8.reshape(8, P, L).transpose(1, 0, 2)),
            np.ascontiguousarray(xe8.reshape(8, P, L).transpose(1, 0, 2)),
        ))

    in_maps = []
    for core in range(N_CORES):
        b, g = divmod(core, 4)
        rq = slice(g * FV, (g + 1) * FV)
        rk = slice(D + g * FV, D + (g + 1) * FV)
        rv = slice(2 * D + g * FV, 2 * D + (g + 1) * FV)
        wqk_t = np.concatenate([qkv_w[rq], qkv_w[rk]], axis=0).T * WS
        w8, we8 = _fp8_split(wqk_t)

        def to_fc(a):
            return np.ascontiguousarray(
                a.reshape(D, 4, P).transpose(1, 0, 2)
                .reshape(4, 8, P, P).transpose(2, 0, 1, 3))

        wv_t = qkv_w[rv].T * WS                       # [D, 256]
        wv8, wve8 = _fp8_split(wv_t)

        def to_v(a):
            return np.ascontiguousarray(a.reshape(8, P, FV).transpose(1, 0, 2))

        wo16 = out_w[:, g * FV:(g + 1) * FV].T.astype(np.float16)  # [256, D]
        wo16 = np.ascontiguousarray(wo16.reshape(2, P, D).transpose(1, 0, 2))

        bqk = np.ascontiguousarray(
            (WS * np.concatenate([qkv_b[rq], qkv_b[rk]])).reshape(4, P).T)
        bv = np.ascontiguousarray((WS * qkv_b[rv]).reshape(1, FV))
        in_maps.append({
            "x8": xs[b][0], "xe8": xs[b][1],
            "wqk8": to_fc(w8), "wqke8": to_fc(we8),
            "wv8": to_v(wv8), "wve8": to_v(wve8),
            "wo16": wo16,
            "bqk": bqk, "bv": bv,
        })

    res = run_bass_kernel_spmd(nc, in_maps, list(range(N_CORES)))
    y = np.empty((B, L, D), dtype=np.float32)
    for b in range(B):
        acc = res.results[b * 4 + 0]["y"].astype(np.float32)
        for g in range(1, 4):
            acc = acc + res.results[b * 4 + g]["y"]
        y[b] = acc / WS
    if np.any(out_b):
        y += out_b
    return y


# revision 36
# speedup vs baseline: 1.0538x; 1.0538x over previous
"""Sliding-window MHA Trainium2 kernel, sharded over 8 NeuronCores.

Problem (hardcoded): B=2, L=2048, D=1024, H=16 heads (hd=64), window
|i-j| <= 256, fp32 I/O.

Sharding: core = b*4 + g (b in 0..1 batches, g in 0..3 head-groups of 4
heads). Each core: QKV projection for its 4 heads, banded attention, and
a partial output projection (its 256 columns of the head concat). Host
sums the partials, removes the dither correction, and rescales.

Numerics (all fp8 uses are residual-corrected except the single q
requantization):
  - Weights are scaled x16 before e4m3 quantization (their natural scale
    sits in e4m3's subnormal range); activations/V/outputs carry the x16
    factor through and the host divides by 256 at the end.
  - QKV projections: fp8 DoubleRow (contraction 2x128/matmul, 0.5
    cyc/col), 3 terms: x8@w8 + xe8@w8 + x8@we8 (x and w residuals).
  - Scores: fp8 DoubleRow per head; the two z-slots compute
    (k8 + ke8).q8, i.e. the k requantization is corrected in-slot for
    free. ke8 is produced by one extra DVE op per k chain. The q
    requantization is the one uncorrected noise source.
  - AV in fp16 (V carries 16x), both heads of a pair in one PSUM bank
    with per-region start/stop; one reciprocal + one stride-0 broadcast
    normalize per pair.
  - Out-projection in fp16 (o stays 16x; host divides by 16): fp8
    here is too noisy for the 2e-2 gate even dithered.
"""

import numpy as np
import ml_dtypes

import concourse.bacc as bacc
import concourse.mybir as mybir
import concourse.tile as tile
from concourse.bass_utils import run_bass_kernel_spmd
from concourse.masks import make_identity

F32 = mybir.dt.float32
F16 = mybir.dt.float16
F8 = mybir.dt.float8e4
E4M3 = ml_dtypes.float8_e4m3
DR = mybir.MatmulPerfMode.DoubleRow

P = 128
L = 2048
D = 1024
NH = 4          # heads per core
HD = 64
FV = 256        # v feature rows per core
WIN = 256
KB = L // P     # 16 k/token blocks
N_CORES = 8
WS = 16.0       # weight/activation scale before fp8
EXP_SCALE = 0.125 / (WS * WS)

C1_LAG = 3
D_LAG = 4


def _window(kb):
    k0 = kb * P
    qlo = max(0, k0 - WIN)
    qhi = min(L, k0 + P + WIN)
    return qlo, qhi - qlo


def _build_nc():
    nc = bacc.Bacc(
        "TRN2", target_bir_lowering=False, debug=False, num_devices=N_CORES
    )
    # x packed as [P, (x8, xe8), dc, L]; qk weights as [P, term, fcx, dc, P]
    # with fcx order (fc0, fc2, fc1, fc3); v weights as [P, term, dc, FV]
    x_d = nc.dram_tensor("xall", [P, 2, 8, L], F8, kind="ExternalInput").ap()
    wqk_d = nc.dram_tensor("wqkall", [P, 2, 4, 8, P], F8,
                           kind="ExternalInput").ap()
    wv_d = nc.dram_tensor("wvall", [P, 2, 8, FV], F8,
                          kind="ExternalInput").ap()
    wo_d = nc.dram_tensor("wo16", [P, 2, D], F16, kind="ExternalInput").ap()
    bqk_d = nc.dram_tensor("bqk", [P, 4], F32, kind="ExternalInput").ap()
    bv_d = nc.dram_tensor("bv", [1, FV], F32, kind="ExternalInput").ap()
    y_d = nc.dram_tensor("y", [L, D], F16, kind="ExternalOutput").ap()

    with tile.TileContext(nc) as tc:
        _emit(nc, tc, x_d, wqk_d, wv_d, wo_d, bqk_d, bv_d, y_d)
    nc.compile()
    return nc


def _emit(nc, tc, x_d, wqk_d, wv_d, wo_d, bqk_d, bv_d, y_d):
    import contextlib

    ctx = contextlib.ExitStack()
    with ctx:
        const = ctx.enter_context(tc.tile_pool(name="const", bufs=1))
        w_pool = ctx.enter_context(tc.tile_pool(name="w", bufs=1))
        qk_pool = ctx.enter_context(tc.tile_pool(name="qk", bufs=1))
        v_pool = ctx.enter_context(tc.tile_pool(name="v", bufs=1))
        x_pool = ctx.enter_context(tc.tile_pool(name="x", bufs=1))
        e_pool = ctx.enter_context(tc.tile_pool(name="e", bufs=8))
        oT_pool = ctx.enter_context(tc.tile_pool(name="oT", bufs=1))
        opr_pool = ctx.enter_context(tc.tile_pool(name="opr", bufs=4))
        rr_pool = ctx.enter_context(tc.tile_pool(name="rr", bufs=6))
        ysb_pool = ctx.enter_context(tc.tile_pool(name="ysb", bufs=4))
        spool = ctx.enter_context(tc.tile_pool(name="spsum", bufs=2, space="PSUM"))
        pqpool = ctx.enter_context(tc.tile_pool(name="pqpsum", bufs=2, space="PSUM"))
        otpool = ctx.enter_context(tc.tile_pool(name="otpsum", bufs=2, space="PSUM"))

        # ---- input DMAs (pipeline-ordered) ------------------------------
        bqk_sb = const.tile([P, 4], F32)
        bv_row = const.tile([1, FV], F32)
        wqk_sb = w_pool.tile([P, 2, 4, 8, P], F8)
        x_sb = x_pool.tile([P, 2, 8, L], F8)
        # startup order: first x chunk (both terms), q/k weights for
        # pair-0 heads (fcx 0:2 = fc0,fc2), rest of weights, rest of x
        sl0 = slice(0, 512)
        nc.sync.dma_start(x_sb[:, 0, 0:4, sl0], x_d[:, 0, 0:4, sl0])
        nc.sync.dma_start(wqk_sb[:, 0, 0, :, :], wqk_d[:, 0, 0, :, :])
        nc.sync.dma_start(x_sb[:, 0, 4:8, sl0], x_d[:, 0, 4:8, sl0])
        nc.sync.dma_start(wqk_sb[:, 1, 0, :, :], wqk_d[:, 1, 0, :, :])
        nc.sync.dma_start(x_sb[:, 1, :, sl0], x_d[:, 1, :, sl0])
        nc.sync.dma_start(bqk_sb[:], bqk_d[:])
        nc.sync.dma_start(bv_row[:], bv_d[:])
        wv_sb = w_pool.tile([P, 2, 8, FV], F8)
        wo_sb = w_pool.tile([P, 2, D], F16)
        for fcx in (1, 2, 3):
            nc.sync.dma_start(wqk_sb[:, 0, fcx, :, :], wqk_d[:, 0, fcx, :, :])
            nc.sync.dma_start(wqk_sb[:, 1, fcx, :, :], wqk_d[:, 1, fcx, :, :])
            if fcx == 2:
                nc.sync.dma_start(wv_sb[:, 0], wv_d[:, 0])
                nc.sync.dma_start(wv_sb[:, 1], wv_d[:, 1])
        for t in range(1, 4):
            sl = slice(t * 512, (t + 1) * 512)
            nc.sync.dma_start(x_sb[:, 0, :, sl], x_d[:, 0, :, sl])
            nc.sync.dma_start(x_sb[:, 1, :, sl], x_d[:, 1, :, sl])
            if t == 2:
                nc.sync.dma_start(wo_sb[:], wo_d[:])

        # ---- constants --------------------------------------------------
        ident_f32 = const.tile([P, P], F32)
        make_identity(nc, ident_f32[:])
        ident = const.tile([P, P], F16)
        nc.vector.tensor_copy(ident[:], ident_f32[:])
        bv_bc = const.tile([P, FV], F32)
        nc.gpsimd.partition_broadcast(bv_bc[:], bv_row[:])

        # Q^T (fp8, 16x): [dims(2 heads x 64), cc, token]
        qT8 = qk_pool.tile([P, 2, L], F8)
        # K^T with z-dim: z0 = k8, z1 = requant residual
        kT8 = qk_pool.tile([P, 2, 2, L], F8)
        v_ext = v_pool.tile([P, KB, NH * (HD + 1)], F16)
        nc.vector.memset(
            v_ext[:].rearrange("p b (h c) -> p b h c", h=NH)[:, :, :, HD:],
            1.0)
        oT = oT_pool.tile([P, 2, L], F16)

        # ---- emission helpers -------------------------------------------
        def qk_chain(fc, t, half=None):
            """12 fp8-DR matmuls (x8@w8, xe8@w8, x8@we8), fp8 bias-add;
            for k chains also the requant residual (z1)."""
            if half is None:
                tsl = slice(t * 512, (t + 1) * 512)
            else:
                tsl = slice(t * 512 + half * 256, t * 512 + half * 256 + 256)
            n = tsl.stop - tsl.start
            pq = pqpool.tile([P, 512], F32, tag="pq", name="pq")
            fcx = {0: 0, 2: 1, 1: 2, 3: 3}[fc]
            # (x8@w8, x8@we8, xe8@w8)
            terms = ((0, 0), (0, 1), (1, 0))
            for i, (xi, wi) in enumerate(terms):
                for d4 in range(4):
                    nc.tensor.matmul(
                        pq[:, 0:n],
                        lhsT=wqk_sb[:, wi, fcx, 2 * d4:2 * d4 + 2, :],
                        rhs=x_sb[:, xi, 2 * d4:2 * d4 + 2, tsl],
                        start=(i == 0 and d4 == 0), stop=(i == 2 and d4 == 3),
                        perf_mode=DR,
                    )
            cc = fc % 2
            if fc < 2:
                nc.vector.tensor_scalar_add(
                    qT8[:, cc, tsl], pq[:, 0:n], bqk_sb[:, fc:fc + 1])
            else:
                nc.vector.tensor_scalar_add(
                    kT8[:, cc, 0, tsl], pq[:, 0:n], bqk_sb[:, fc:fc + 1])
                # z1 = (pq + bias) - k8   (requant residual)
                nc.vector.scalar_tensor_tensor(
                    kT8[:, cc, 1, tsl], pq[:, 0:n], bqk_sb[:, fc:fc + 1],
                    kT8[:, cc, 0, tsl],
                    mybir.AluOpType.add, mybir.AluOpType.subtract)

        def v_chain(tb):
            tsl = slice(tb * P, (tb + 1) * P)
            pv = pqpool.tile([P, 512], F32, tag="pq", name="pv")
            terms = ((0, 0), (0, 1), (1, 0))
            for i, (xi, wi) in enumerate(terms):
                for d4 in range(4):
                    nc.tensor.matmul(
                        pv[:, 0:FV],
                        lhsT=x_sb[:, xi, 2 * d4:2 * d4 + 2, tsl],
                        rhs=wv_sb[:, wi, 2 * d4:2 * d4 + 2, :],
                        start=(i == 0 and d4 == 0), stop=(i == 2 and d4 == 3),
                        perf_mode=DR,
                    )
            nc.vector.tensor_add(
                v_ext[:, tb, :].rearrange("p (h c) -> p h c", h=NH)[:, :, 0:HD],
                pv[:, 0:FV].rearrange("p (h c) -> p h c", h=NH),
                bv_bc[:].rearrange("p (h c) -> p h c", h=NH),
            )

        def phase_b(h, kb, e_tiles):
            """Scores for one head: fp8-DR, z = (k8, k-residual)."""
            qlo, w = _window(kb)
            if h == 0:
                e4 = e_pool.tile([P, NH, 640], F16, tag="e", name="e4")
                e_tiles[kb] = e4
            else:
                e4 = e_tiles[kb]
            s_ps = spool.tile([P, 1024], F32, tag="s", name="s_ps")
            cc, hh = h // 2, h % 2
            base = 64 * hh
            if w == 640:
                pieces = [(0, 0, 320), (320, 512, 320)]
            else:
                pieces = [(0, 0, w)]
            lhsT = kT8[base:base + 64, cc, :, kb * P:(kb + 1) * P]
            for qoff, poff, pw in pieces:
                rhs = (qT8[base:base + 64, cc, qlo + qoff:qlo + qoff + pw]
                       .unsqueeze(1).broadcast_to([64, 2, pw]))
                nc.tensor.matmul(
                    s_ps[:, poff:poff + pw],
                    lhsT=lhsT, rhs=rhs, start=True, stop=True,
                    perf_mode=DR, skip_group_check=True,
                )
            if w == 640:
                src = s_ps[:].rearrange("p (g c) -> p g c", g=2)[:, :, 0:320]
                dst = e4[:, h, :].rearrange("p (g c) -> p g c", g=2)
            else:
                src = s_ps[:, 0:w]
                dst = e4[:, h, 0:w]
            nc.scalar.activation(
                dst, src, mybir.ActivationFunctionType.Exp, scale=EXP_SCALE)

        def masks(kb, pair, e_tiles):
            e4 = e_tiles[kb]
            qlo, w = _window(kb)
            hs = slice(2 * pair, 2 * pair + 2)
            if kb >= 2:
                nc.gpsimd.affine_select(
                    out=e4[:, hs, 0:P], in_=e4[:, hs, 0:P],
                    compare_op=mybir.AluOpType.is_ge, fill=0.0,
                    base=0, pattern=[[0, 2], [1, P]], channel_multiplier=-1,
                )
            if kb <= KB - 3:
                nc.gpsimd.affine_select(
                    out=e4[:, hs, w - P:w], in_=e4[:, hs, w - P:w],
                    compare_op=mybir.AluOpType.is_ge, fill=0.0,
                    base=0, pattern=[[0, 2], [-1, P]], channel_multiplier=1,
                )

        def phase_c(pair, qt, e_tiles):
            """AV (pair in one bank), recip + stride-0 normalize, fp16
            transpose, dithered fp8 oT write."""
            kbs = range(max(0, qt - 2), min(KB, qt + 3))
            ot = otpool.tile([P, 512], F32, tag="ot", name="ot")
            for hh in range(2):
                h = 2 * pair + hh
                for i, kb in enumerate(kbs):
                    qlo, _ = _window(kb)
                    off = qt * P - qlo
                    nc.tensor.matmul(
                        ot[:, hh * 65:hh * 65 + 65],
                        lhsT=e_tiles[kb][:, h, off:off + P],
                        rhs=v_ext[:, kb, h * 65:h * 65 + 65],
                        start=(i == 0), stop=(i == len(kbs) - 1),
                        skip_group_check=True,
                    )
            o_pair = ot[:, 0:130].rearrange("p (h c) -> p h c", h=2)
            rr = rr_pool.tile([P, 2], F32, tag="rr", name="rr")
            nc.vector.reciprocal(rr[:], o_pair[:, :, HD:HD + 1])
            opr = opr_pool.tile([P, P], F16, tag="opr", name="opr")
            nc.vector.tensor_mul(
                opr[:].rearrange("p (h c) -> p h c", h=2),
                o_pair[:, :, 0:HD],
                rr[:].rearrange("p (h o) -> p h o", h=2).broadcast_to([P, 2, HD]),
            )
            t_ps = ot[:, 256:320].bitcast(F16)
            nc.tensor.transpose(t_ps, opr[:], ident[:])
            nc.vector.tensor_copy(oT[:, pair, qt * P:(qt + 1) * P], t_ps)

        def phase_d_mm(qt):
            y_list = []
            for half in range(2):
                y_ps = pqpool.tile([P, 512], F32, tag="pq", name="y_ps")
                hsl = slice(half * 512, (half + 1) * 512)
                for cc in range(2):
                    nc.tensor.matmul(
                        y_ps[:],
                        lhsT=oT[:, cc, qt * P:(qt + 1) * P],
                        rhs=wo_sb[:, cc, hsl],
                        start=(cc == 0), stop=(cc == 1),
                    )
                y_list.append(y_ps)
            return y_list

        def phase_d_copy(qt, y_list, step):
            y_sb = ysb_pool.tile([P, D], F16, tag="ysb", name="y_sb")
            tail = qt >= KB - 2
            for half, y_ps in enumerate(y_list):
                hsl = slice(half * 512, (half + 1) * 512)
                if tail:
                    q0 = slice(half * 512, half * 512 + 256)
                    q1 = slice(half * 512 + 256, half * 512 + 512)
                    nc.vector.tensor_copy(y_sb[:, q0], y_ps[:, 0:256])
                    nc.scalar.copy(y_sb[:, q1], y_ps[:, 256:512])
                    nc.sync.dma_start(y_d[qt * P:(qt + 1) * P, hsl],
                                      y_sb[:, hsl])
                elif step >= 16:
                    # B phases done: ACT is free, DVE is the tail pacer
                    nc.scalar.copy(y_sb[:, hsl], y_ps[:])
                elif step >= 10 or half == 0:
                    nc.vector.tensor_copy(y_sb[:, hsl], y_ps[:])
                else:
                    nc.scalar.copy(y_sb[:, hsl], y_ps[:])
            if not tail:
                nc.sync.dma_start(y_d[qt * P:(qt + 1) * P, :], y_sb[:])

        # ---- schedule ---------------------------------------------------
        proj_a, proj_b = {}, {}
        proj_a[1] = [lambda: qk_chain(0, 1), lambda: qk_chain(2, 1)]
        proj_b[1] = [lambda: qk_chain(1, 1), lambda: qk_chain(3, 1)]
        proj_a[3] = [lambda: qk_chain(0, 2)]
        proj_b[3] = [lambda: qk_chain(2, 2)]
        proj_a[4] = [lambda: qk_chain(1, 2)]
        proj_b[4] = [lambda: qk_chain(3, 2)]
        proj_a[6] = [lambda: qk_chain(0, 3)]
        proj_b[6] = [lambda: qk_chain(2, 3)]
        proj_a[7] = [lambda: qk_chain(1, 3)]
        proj_b[7] = [lambda: qk_chain(3, 3)]
        # v blocks: emitted in the early slot (before phase_c of the
        # step), respecting wv DMA arrival (~step 2)
        v_sched_a = {3: [2], 4: [3, 4]}
        for s in range(5, KB):
            v_sched_a[s] = [s]
        v_sched_b = {2: [0, 1]}

        # prologue: wave 0 chains
        for fc in (0, 2, 1, 3):
            qk_chain(fc, 0)

        e_tiles = {}
        for step in range(KB + C1_LAG):
            kb = step if step < KB else None
            qt = step - C1_LAG
            qt2 = step - D_LAG
            if kb is not None:
                phase_b(0, kb, e_tiles)
                phase_b(1, kb, e_tiles)
                masks(kb, 0, e_tiles)
            for f in proj_a.get(step, ()):
                f()
            for n in v_sched_a.get(step, ()):
                v_chain(n)
            early_d = False
            y_list = None
            if qt in range(KB):
                phase_c(0, qt, e_tiles)
            if early_d and qt2 in range(KB):
                y_list = phase_d_mm(qt2)
            if kb is not None:
                phase_b(2, kb, e_tiles)
                phase_b(3, kb, e_tiles)
                masks(kb, 1, e_tiles)
            for f in proj_b.get(step, ()):
                f()
            for n in v_sched_b.get(step, ()):
                v_chain(n)
            if qt in range(KB):
                phase_c(1, qt, e_tiles)
            if qt2 in range(KB):
                if y_list is None:
                    y_list = phase_d_mm(qt2)
                phase_d_copy(qt2, y_list, step)
            if qt == KB - 1:
                phase_d_copy(KB - 1, phase_d_mm(KB - 1), step)


_NC_CACHE = None


def _get_nc():
    global _NC_CACHE
    if _NC_CACHE is None:
        _NC_CACHE = _build_nc()
    return _NC_CACHE


def _fp8_split(a):
    hi = a.astype(E4M3)
    lo = (a - hi.astype(np.float32)).astype(E4M3)
    return hi, lo


def kernel(x, qkv_w, qkv_b, out_w, out_b):
    x = np.asarray(x, dtype=np.float32)
    qkv_w = np.asarray(qkv_w, dtype=np.float32)
    qkv_b = np.asarray(qkv_b, dtype=np.float32)
    out_w = np.asarray(out_w, dtype=np.float32)
    out_b = np.asarray(out_b, dtype=np.float32)
    B = x.shape[0]
    assert x.shape == (B, L, D) and B * 4 == N_CORES

    nc = _get_nc()

    xs = []
    for b in range(B):
        xt = np.ascontiguousarray(x[b].T)            # [D, L]
        x8, xe8 = _fp8_split(xt)
        xall = np.stack([x8.reshape(8, P, L), xe8.reshape(8, P, L)])
        xs.append(np.ascontiguousarray(xall.transpose(2, 0, 1, 3)))

    in_maps = []
    for core in range(N_CORES):
        b, g = divmod(core, 4)
        rq = slice(g * FV, (g + 1) * FV)
        rk = slice(D + g * FV, D + (g + 1) * FV)
        rv = slice(2 * D + g * FV, 2 * D + (g + 1) * FV)
        wqk_t = np.concatenate([qkv_w[rq], qkv_w[rk]], axis=0).T * WS
        w8, we8 = _fp8_split(wqk_t)
        FCX = [0, 2, 1, 3]

        def to_fc(a):
            # [D, 512] -> [4 fc][8 dc][P][P] -> fcx order -> [P,4,8,P]
            r = (a.reshape(D, 4, P).transpose(1, 0, 2)
                 .reshape(4, 8, P, P))[FCX]
            return r.transpose(2, 0, 1, 3)

        wqk_all = np.ascontiguousarray(
            np.stack([to_fc(w8), to_fc(we8)], axis=1))

        wv_t = qkv_w[rv].T * WS                       # [D, 256]
        wv8, wve8 = _fp8_split(wv_t)

        def to_v(a):
            return a.reshape(8, P, FV).transpose(1, 0, 2)

        wv_all = np.ascontiguousarray(
            np.stack([to_v(wv8), to_v(wve8)], axis=1))

        wo16 = out_w[:, g * FV:(g + 1) * FV].T.astype(np.float16)  # [256, D]
        wo16 = np.ascontiguousarray(wo16.reshape(2, P, D).transpose(1, 0, 2))

        bqk = np.ascontiguousarray(
            (WS * np.concatenate([qkv_b[rq], qkv_b[rk]])).reshape(4, P).T)
        bv = np.ascontiguousarray((WS * qkv_b[rv]).reshape(1, FV))
        in_maps.append({
            "xall": xs[b], "wqkall": wqk_all, "wvall": wv_all,
            "wo16": wo16,
            "bqk": bqk, "bv": bv,
        })

    res = run_bass_kernel_spmd(nc, in_maps, list(range(N_CORES)))
    y = np.empty((B, L, D), dtype=np.float32)
    for b in range(B):
        acc = res.results[b * 4 + 0]["y"].astype(np.float32)
        for g in range(1, 4):
            acc = acc + res.results[b * 4 + g]["y"]
        y[b] = acc / WS
    if np.any(out_b):
        y += out_b
    return y
